# revision 10
# baseline (speedup 1.0000x reference)
"""Trainium2 Bass kernel for nn_CNN_88098369175780.

Strategy (8 NeuronCores, ONE NEFF launch, one tiny AllGather):
  Sequence-parallel attention: each core owns a 514-wide q-slice (512 + 2
  halo columns so the conv stack needs no cross-core halo).  The T x T
  matrices are never materialized in HBM; scores are computed in transposed
  orientation (keys on partitions).  Softmax shift uses the algebraic upper
  bound 6*sum(Q) (K <= 6, Q >= 0) folded in as an extra contraction row.

  fp8 fast path: Q/K/V quantized to e4m3, exp(scores) written as e5m2; the
  scores and A@V matmuls run in DoubleRow perf mode (2 contraction rows per
  PE pass = 2x throughput).  A tiny eps (2^-14) is injected into the softmax
  denominator via an extra fp8 matmul so fully-underflowed q columns divide
  by eps instead of 0 (their wrong-but-finite values are diluted to nothing
  by the conv stack; verified numerically at 1e-4 rel err).

  wavP @ (eeg2.T @ wavP) is reassociated through the 16x16 Gram matrix.
  Each core runs conv0-conv2 on its aligned local slice, then a 3.8KB
  AllGather collects the 8 conv2 maps and every core redundantly computes
  conv3 + FC head -> [42, 2].
"""
import contextlib
import ctypes
import os
import sys
import types

import numpy as np

for _p in ('/root/.axon_site', '/root/.axon_site/_ro/trn_rl_repo',
           '/root/.axon_site/_ro/pypackages', '/opt/trn_rl_repo'):
    if os.path.isdir(_p) and _p not in sys.path:
        sys.path.append(_p)

import ml_dtypes
import concourse.bacc as bacc
import concourse.tile as tile
import concourse.mybir as mybir
from concourse.bass_utils import run_bass_kernel_spmd

f32 = mybir.dt.float32
bf16 = mybir.dt.bfloat16
f8e4 = mybir.dt.float8e4
f8e5 = mybir.dt.float8e5
AF = mybir.ActivationFunctionType
ALU = mybir.AluOpType
DR = mybir.MatmulPerfMode.DoubleRow
BF = ml_dtypes.bfloat16
E4 = ml_dtypes.float8_e4m3fn
E5 = ml_dtypes.float8_e5m2

T = 4096
NC = 8
QN = 514


# ---------------------------------------------------------------- NTFF shim
def _install_ntff_shim():
    name = "antenv.axon_hooks"
    if name in sys.modules:
        return
    so_path = "/opt/axon/libaxon_pjrt.so"
    hook = None
    if os.path.exists(so_path):
        lib = ctypes.CDLL(so_path)
        if hasattr(lib, "axon_start_nrt_profile"):
            lib.axon_start_nrt_profile.argtypes = [
                ctypes.POINTER(ctypes.c_int64), ctypes.c_size_t]
            lib.axon_start_nrt_profile.restype = ctypes.c_int64
            lib.axon_stop_nrt_profile.argtypes = [ctypes.c_char_p]
            lib.axon_stop_nrt_profile.restype = ctypes.c_int64

            @contextlib.contextmanager
            def _hook(output_dir, device_ids):
                import jax
                jax.devices()
                if device_ids:
                    ids = (ctypes.c_int64 * len(device_ids))(*device_ids)
                    rc = lib.axon_start_nrt_profile(ids, len(device_ids))
                else:
                    rc = lib.axon_start_nrt_profile(None, 0)
                if rc != 0:
                    raise RuntimeError(f"axon_start_nrt_profile rc={rc}")
                try:
                    yield
                finally:
                    n = lib.axon_stop_nrt_profile(str(output_dir).encode())
                    if n < 0:
                        raise RuntimeError(f"axon_stop_nrt_profile rc={n}")
            hook = _hook
    mod = types.ModuleType(name)
    mod._hook = hook
    mod.set_axon_ntff_profile_hook = lambda h: setattr(mod, "_hook", h)
    mod.get_axon_ntff_profile_hook = lambda: mod._hook
    sys.modules[name] = mod


_install_ntff_shim()


# ------------------------------------------------------------- host consts
def build_consts(x, cm1_W, cm1_b, cm2_W, cm2_b, cw0, cw1, cw2, cw3, cb,
                 fc1_W, fc1_b, fc2_W, fc2_b):
    F = np.float32
    x = np.asarray(x, F)
    eeg2 = np.ascontiguousarray(x[0, 0, 1:-1, :]).astype(F)
    wavA = np.ascontiguousarray(x[0, 0, 0, :]).astype(F)
    wavB = np.ascontiguousarray(x[0, 0, -1, :]).astype(F)
    cm1_W = np.asarray(cm1_W, F); cm1_b = np.asarray(cm1_b, F)
    cm2_W = np.asarray(cm2_W, F); cm2_b = np.asarray(cm2_b, F)
    cw0 = np.asarray(cw0, F); cw1 = np.asarray(cw1, F)
    cw2 = np.asarray(cw2, F); cw3 = np.asarray(cw3, F); cb = np.asarray(cb, F)
    fc1_W = np.asarray(fc1_W, F); fc1_b = np.asarray(fc1_b, F)
    fc2_W = np.asarray(fc2_W, F); fc2_b = np.asarray(fc2_b, F)

    c = {}
    E_aug = np.concatenate([eeg2, np.ones((1, T), F)], 0)          # [17, T]
    wb49 = np.zeros((49, T), F)
    wb49[0:16] = wavA[None, :]; wb49[32:48] = wavB[None, :]
    wb49[16] = 1.0; wb49[48] = 1.0
    c['EWB'] = np.concatenate([E_aug, wb49], 0).astype(BF)         # [66, T]

    et = np.transpose(eeg2.reshape(16, 32, 128), (2, 1, 0))
    ET_dup = np.concatenate([et, et], axis=2).reshape(128, 1024)
    wa = wavA.reshape(32, 128).T[:, :, None]
    wb = wavB.reshape(32, 128).T[:, :, None]
    wav_exp = np.concatenate(
        [np.repeat(wa, 16, 2), np.repeat(wb, 16, 2)], axis=2).reshape(128, 1024)
    c['ETW'] = np.concatenate([ET_dup, wav_exp], 1).astype(BF)     # [128, 2048]

    # packed small consts [49, 758]: cols 0:244 as v1, cols 244:758 E_slice
    lk = np.zeros((49, 48), F)
    lk[0:16, 0:16] = cm1_W[1].T; lk[16, 0:16] = cm1_b[1]
    lk[32:48, 32:48] = cm2_W[1].T; lk[48, 32:48] = cm2_b[1]
    rv = np.zeros((49, 66), F)
    rv[0:16, 0:16] = cm1_W[2].T; rv[16, 0:16] = cm1_b[2]; rv[16, 32] = 1.0
    rv[32:48, 33:49] = cm2_W[2].T; rv[48, 33:49] = cm2_b[2]; rv[48, 65] = 1.0
    lq = np.zeros((17, 48), F)
    lq[0:16, 0:16] = cm1_W[0].T; lq[16, 0:16] = cm1_b[0]
    lq[0:16, 32:48] = cm2_W[0].T; lq[16, 32:48] = cm2_b[0]
    g2i = np.zeros((17, 49), F)
    g2i[16, 16] = 1.0; g2i[16, 48] = 1.0
    W3A = np.concatenate([cm1_W[3].T, cm1_b[3][None, :]], 0)
    W3B = np.concatenate([cm2_W[3].T, cm2_b[3][None, :]], 0)
    cpk = np.zeros((NC, 49, 758), F)
    cpk[:, 0:49, 0:48] = lk
    cpk[:, 0:49, 48:114] = rv
    cpk[:, 0:17, 114:163] = g2i
    cpk[:, 0:17, 163:211] = lq
    cpk[:, 0:17, 211:227] = W3A
    cpk[:, 0:17, 227:243] = W3B
    cpk[:, :, 243] = 1.0
    for ci in range(NC):
        n = min(QN, T - 512 * ci)
        cpk[ci, 0:17, 244:244 + n] = E_aug[:, 512 * ci:512 * ci + n]
        if n < QN:
            cpk[ci, 0:17, 244 + n:758] = 0.0
    c['CONSTX'] = cpk.astype(BF)

    # fp8 consts: rows 0:16 col0 = 1 (ones16); row 0 cols 0:512 = 1 (eps rhs);
    # row 16 cols 0:4096 = -6; row 16 cols 4096:8192 = 0
    f8 = np.zeros((17, 8192), np.float64)
    f8[0:16, 0] = 1.0
    f8[0, 0:512] = 1.0
    f8[16, 0:4096] = -6.0
    c['F8'] = f8.astype(E4)
    c['F8B'] = np.ones((48, 4), np.float64).astype(E4)
    f8e5c = np.zeros((1, 64), np.float64)
    f8e5c[0, 32] = 2.0 ** -14
    c['F8E5'] = f8e5c.astype(E5)

    def y48row(origH):
        if 16 <= origH < 32:
            return origH - 16
        if origH < 16:
            return origH + 16
        return origH
    c0 = np.zeros((3, 49, 120), F)
    for dw in range(3):
        for cch in range(5):
            for h in range(24):
                m = cch * 24 + h
                for dh in range(2):
                    c0[dw, y48row(2 * h + dh), m] += cw0[cch, 0, dh, dw]
                if dw == 0:
                    c0[dw, 48, m] += cb[0][cch]
    c1 = np.zeros((4, 121, 60), F)
    for dw in range(4):
        for cch in range(5):
            for h in range(12):
                m = cch * 12 + h
                for cin in range(5):
                    for dh in range(2):
                        c1[dw, cin * 24 + 2 * h + dh, m] += cw1[cch, cin, dh, dw]
                if dw == 0:
                    c1[dw, 120, m] += cb[1][cch]
    c2 = np.zeros((4, 61, 30), F)
    for dw in range(4):
        for cch in range(5):
            for h in range(6):
                m = cch * 6 + h
                for cin in range(5):
                    for dh in range(2):
                        c2[dw, cin * 12 + 2 * h + dh, m] += cw2[cch, cin, dh, dw]
                if dw == 0:
                    c2[dw, 60, m] += cb[2][cch]
    cvw = np.zeros((121, 720), F)
    for dw in range(3):
        cvw[0:49, 120 * dw:120 * dw + 120] = c0[dw]
    for dw in range(4):
        cvw[0:121, 360 + 60 * dw:360 + 60 * dw + 60] = c1[dw]
    for dw in range(4):
        cvw[0:61, 600 + 30 * dw:600 + 30 * dw + 30] = c2[dw]
    c['CONVW'] = cvw.astype(BF)

    # head consts f32: c3w 4x[31,15] cols 0:60, f1w [31,15] cols 60:75,
    # f2w [16,2] cols 75:77
    c3 = np.zeros((4, 31, 15), F)
    for dw in range(4):
        for cch in range(5):
            for h in range(3):
                m = cch * 3 + h
                for cin in range(5):
                    for dh in range(2):
                        c3[dw, cin * 6 + 2 * h + dh, m] += cw3[cch, cin, dh, dw]
                if dw == 0:
                    c3[dw, 30, m] += cb[3][cch]
    hw = np.zeros((31, 77), F)
    for dw in range(4):
        hw[:, 15 * dw:15 * dw + 15] = c3[dw]
    hw[:, 60:75] = np.concatenate([fc1_W.T, fc1_b[None, :]], 0)
    w_d = np.stack([fc2_W[0] - fc2_W[1], fc2_W[1] - fc2_W[0]], 1)
    b_d = np.array([fc2_b[0] - fc2_b[1], fc2_b[1] - fc2_b[0]], F)
    hw[0:16, 75:77] = np.concatenate([w_d, b_d[None, :]], 0)
    c['HEADW'] = hw
    c['HONES'] = np.ones((1, 256), F)
    return c


# ---------------------------------------------------------------- kernel
def _build():
    nc = bacc.Bacc("TRN2", target_bir_lowering=False, debug=False,
                   num_devices=NC)
    dt = nc.dram_tensor
    a = {
        'ETW':    dt('ETW',    [128, 2048], bf16, kind="ExternalInput").ap(),
        'EWB':    dt('EWB',    [66, T],     bf16, kind="ExternalInput").ap(),
        'CONSTX': dt('CONSTX', [49, 758],   bf16, kind="ExternalInput").ap(),
        'CONVW':  dt('CONVW',  [121, 720],  bf16, kind="ExternalInput").ap(),
        'F8':     dt('F8',     [17, 8192],  f8e4, kind="ExternalInput").ap(),
        'F8E5':   dt('F8E5',   [1, 64],     f8e5, kind="ExternalInput").ap(),
        'F8B':    dt('F8B',    [48, 4],     f8e4, kind="ExternalInput").ap(),
        'HEADW':  dt('HEADW',  [31, 77],    f32, kind="ExternalInput").ap(),
        'HONES':  dt('HONES',  [1, 256],    f32, kind="ExternalInput").ap(),
        'out':    dt('out',    [42, 2],     f32, kind="ExternalOutput").ap(),
        'scr':    dt('scratch', [15, 84],   f32).ap(),
    }

    with tile.TileContext(nc) as tc:
        with tc.tile_pool(name="const", bufs=1) as cp, \
             tc.tile_pool(name="work", bufs=2) as wp, \
             tc.tile_pool(name="exps", bufs=3) as ep, \
             tc.tile_pool(name="dram", bufs=1, space="DRAM") as dp, \
             tc.tile_pool(name="psumP", bufs=2, space="PSUM") as psP, \
             tc.tile_pool(name="psumUA", bufs=1, space="PSUM") as psUA, \
             tc.tile_pool(name="psumUB", bufs=1, space="PSUM") as psUB, \
             tc.tile_pool(name="psumH", bufs=1, space="PSUM") as psH, \
             tc.tile_pool(name="psumS", bufs=1, space="PSUM") as psS:

            # ---- input loads, spread across engine queues
            ETW = cp.tile([128, 2048], bf16, tag="ETW")
            EWBa = cp.tile([17, T], bf16, tag="EWBa")
            EWBb = cp.tile([49, T], bf16, tag="EWBb")
            CONSTX = cp.tile([49, 758], bf16, tag="CONSTX")
            CONVW = cp.tile([121, 720], bf16, tag="CONVW")
            F8 = cp.tile([17, 8192], f8e4, tag="F8")
            F8E5 = cp.tile([1, 64], f8e5, tag="F8E5")
            F8B = cp.tile([48, 4], f8e4, tag="F8B")
            HEADW = cp.tile([31, 77], f32, tag="HEADW")
            HONES = cp.tile([1, 256], f32, tag="HONES")
            nc.sync.dma_start(ETW[:], a['ETW'][:])
            nc.gpsimd.dma_start(EWBa[:], a['EWB'][0:17, :])
            nc.gpsimd.dma_start(EWBb[:], a['EWB'][17:66, :])
            nc.scalar.dma_start(CONSTX[:], a['CONSTX'][:])
            nc.gpsimd.dma_start(F8[:], a['F8'][:])
            nc.sync.dma_start(CONVW[:], a['CONVW'][:])
            nc.scalar.dma_start(F8E5[:], a['F8E5'][:])
            nc.scalar.dma_start(F8B[:], a['F8B'][:])
            nc.gpsimd.dma_start(HEADW[:], a['HEADW'][:])
            nc.gpsimd.dma_start(HONES[:], a['HONES'][:])

            lhsK = CONSTX[0:49, 0:48]
            rhsV49 = CONSTX[0:49, 48:114]
            G2 = CONSTX[0:17, 114:163]
            lhsQ = CONSTX[0:17, 163:211]
            W3A = CONSTX[0:17, 211:227]
            W3B = CONSTX[0:17, 227:243]
            ones16 = CONSTX[0:16, 243:244]
            E_sl = CONSTX[0:17, 244:758]
            ones8 = F8[0:16, 0:1]
            neg6row = F8[16:17, 0:4096]
            zrow = F8[16:17, 4096:8192]
            onesrow8 = F8[0:1, 0:512]
            epsW = F8E5[0:1, 0:33]

            # ---- 1. wavPT = ET_dup * wav_exp
            wavPT = cp.tile([128, 1024], bf16, tag="wavPT")
            nc.vector.tensor_tensor(wavPT[:], ETW[:, 0:1024],
                                    ETW[:, 1024:2048], op=ALU.mult)

            # ---- 2. Gram matrix -> G2 rows 0:16
            gps = psS.tile([16, 32], f32, tag="S")
            for g in range(32):
                nc.tensor.matmul(gps[:], ETW[:, 32 * g:32 * g + 16],
                                 wavPT[:, 32 * g:32 * g + 32],
                                 start=(g == 0), stop=(g == 31))
            nc.vector.tensor_copy(G2[0:16, 0:16], gps[:, 0:16])
            nc.vector.tensor_copy(G2[0:16, 32:48], gps[:, 16:32])

            # ---- 3+4. wavP2 and KT (fp8 staging), interleaved per chunk
            wavP2 = cp.tile([49, T], bf16, tag="wavP2")
            KT8s = cp.tile([49, T], f8e4, tag="KT8s")
            for j in range(8):
                geps = psP.tile([49, 512], f32, tag="P")
                nc.tensor.matmul(geps[:], G2, EWBa[:, 512 * j:512 * (j + 1)],
                                 start=True, stop=True)
                nc.vector.tensor_tensor(wavP2[:, 512 * j:512 * (j + 1)], geps[:],
                                        EWBb[:, 512 * j:512 * (j + 1)], op=ALU.mult)
                kps = psP.tile([48, 512], f32, tag="P")
                nc.tensor.matmul(kps[:], lhsK, wavP2[:, 512 * j:512 * (j + 1)],
                                 start=True, stop=True)
                nc.vector.tensor_scalar(KT8s[0:48, 512 * j:512 * (j + 1)], kps[:],
                                        0.0, 6.0, ALU.max, ALU.min)

            # ---- 5. Q (fp8) + sumQ
            QT8s = cp.tile([49, QN], f8e4, tag="QT8s")
            qp1 = psS.tile([48, 512], f32, tag="S")
            halo = psH.tile([128, 512], f32, tag="H")
            qp2 = halo[0:48, 136:138]
            nc.tensor.matmul(qp1[:], lhsQ, E_sl[:, 0:512], start=True, stop=True)
            nc.tensor.matmul(qp2, lhsQ, E_sl[:, 512:QN], start=True, stop=True)
            nc.vector.tensor_scalar(QT8s[0:48, 0:512], qp1[:], 0.0, 6.0,
                                    ALU.max, ALU.min)
            nc.vector.tensor_scalar(QT8s[0:48, 512:QN], qp2, 0.0, 6.0,
                                    ALU.max, ALU.min)
            sumQ8 = {}
            for bi, lo in ((0, 0), (1, 32)):
                sq1 = psS.tile([1, 512], f32, tag="S")
                sq2 = halo[0:1, 140:142]
                onesb = F8B[lo:lo + 16, 0:1]
                nc.tensor.matmul(sq1[:], onesb, QT8s[lo:lo + 16, 0:512],
                                 start=True, stop=True)
                nc.tensor.matmul(sq2, onesb, QT8s[lo:lo + 16, 512:QN],
                                 start=True, stop=True)
                sq8 = wp.tile([1, QN], f8e4, tag=f"sq8{bi}")
                nc.vector.tensor_copy(sq8[:, 0:512], sq1[:])
                nc.vector.tensor_copy(sq8[:, 512:QN], sq2)
                sumQ8[bi] = sq8

            nc.scalar.dma_start(KT8s[16:17, :], neg6row)
            nc.scalar.dma_start(KT8s[48:49, :], neg6row)
            nc.scalar.dma_start(QT8s[16:17, :], sumQ8[0][:])
            nc.scalar.dma_start(QT8s[48:49, :], sumQ8[1][:])

            # ---- 6. DoubleRow repack DMAs (partition moves)
            # KT8x [9, 2*4096]: g0 = feats 0:9, g1 = feats 9:16 + (-6) + pad0
            KT8a = cp.tile([9, 2 * T], f8e4, tag="KT8a")
            KT8b = cp.tile([9, 2 * T], f8e4, tag="KT8b")
            QT8a = cp.tile([9, 2 * 528], f8e4, tag="QT8a")
            QT8b = cp.tile([9, 2 * 528], f8e4, tag="QT8b")
            KT8 = {0: KT8a, 1: KT8b}
            QT8 = {0: QT8a, 1: QT8b}
            for bi, lo in ((0, 0), (1, 32)):
                kt, qt = KT8[bi], QT8[bi]
                nc.sync.dma_start(kt[0:9, 0:T], KT8s[lo:lo + 9, :])
                nc.gpsimd.dma_start(kt[0:7, T:2 * T], KT8s[lo + 9:lo + 16, :])
                nc.scalar.dma_start(kt[7:8, T:2 * T], neg6row)
                nc.scalar.dma_start(kt[8:9, T:2 * T], zrow)
                nc.sync.dma_start(qt[0:9, 0:QN], QT8s[lo:lo + 9, :])
                nc.gpsimd.dma_start(qt[0:7, 528:528 + QN], QT8s[lo + 9:lo + 16, :])
                nc.scalar.dma_start(qt[7:8, 528:528 + QN], sumQ8[bi][:])
                nc.scalar.dma_start(qt[8:9, 528:528 + QN], zrow[0:1, 0:QN])

            # ---- 7. V (fp8, DoubleRow-ready layout [128, g, b, 33])
            Vt = cp.tile([128, 32 * 192], f8e4, tag="Vt")
            for g in range(32):
                vps = psP.tile([128, 66], f32, tag="P")
                nc.tensor.matmul(vps[:], wavP2[:, 128 * g:128 * (g + 1)],
                                 rhsV49, start=True, stop=True)
                # A half -> cols +0:33, B half -> cols +48:81 of this 96-block
                dst = Vt[:, 96 * g:96 * g + 96].rearrange(
                    "p (b f) -> p b f", b=2)[:, :, 0:33]
                nc.vector.tensor_scalar(dst, vps[:].rearrange(
                    "p (b f) -> p b f", b=2), 0.0, 6.0, ALU.max, ALU.min)

            def vt_pair(p, bi):
                # [128, 2, 33]: chunks (2p, 2p+1), block bi; group step 96 (16-aligned)
                return Vt[:].rearrange("p (pp g f) -> p pp g f", g=2, f=96)[
                    :, p, :, 48 * bi:48 * bi + 33]

            def vt_chunk(g, bi):
                # [128, 33]: single chunk g, block bi (for non-DR halo U)
                return Vt[:].rearrange("p (pp g f) -> p pp g f", g=2, f=96)[
                    :, g // 2, g % 2, 48 * bi:48 * bi + 33]

            # ---- y48 assembly target
            y48 = cp.tile([49, QN], bf16, tag="y48")
            nc.sync.dma_start(y48[0:16, :], a['CONSTX'][0:16, 244:758])
            nc.sync.dma_start(y48[48:49, :], a['CONSTX'][16:17, 244:758])

            # ---- 8. pair loops (A staggered ahead of B)
            UA = psUA.tile([33, 512], f32, tag="UA")
            UB = psUB.tile([33, 512], f32, tag="UB")

            def emit_eps(U, uh):
                nc.tensor.matmul(U[:, 0:512], epsW, onesrow8,
                                 start=True, stop=False)
                nc.tensor.matmul(uh, epsW, onesrow8[0:1, 0:2],
                                 start=True, stop=False)

            def emit_pair(bi, U, p):
                kt = KT8[bi].rearrange("p (g t) -> p g t", g=2)
                qt = QT8[bi].rearrange("p (g t) -> p g t", g=2)  # t=528
                g0, g1 = 2 * p, 2 * p + 1
                pair = psP.tile([128, 1024], f32, tag="P")
                nc.tensor.matmul(pair[:, 0:512],
                                 kt[:, :, 128 * g0:128 * g0 + 128],
                                 qt[:, :, 0:512], start=True, stop=True,
                                 perf_mode=DR)
                lo = 32 * bi
                nc.tensor.matmul(halo[:, 64 * bi + 4 * p:64 * bi + 4 * p + 2],
                                 KT8s[lo:lo + 17, 128 * g0:128 * g0 + 128],
                                 QT8s[lo:lo + 17, 512:QN], start=True, stop=True)
                nc.tensor.matmul(pair[:, 512:1024],
                                 kt[:, :, 128 * g1:128 * g1 + 128],
                                 qt[:, :, 0:512], start=True, stop=True,
                                 perf_mode=DR)
                nc.tensor.matmul(halo[:, 64 * bi + 4 * p + 2:64 * bi + 4 * p + 4],
                                 KT8s[lo:lo + 17, 128 * g1:128 * g1 + 128],
                                 QT8s[lo:lo + 17, 512:QN], start=True, stop=True)
                ex = ep.tile([128, 1024], f8e5, tag="ex")
                nc.scalar.activation(ex[:], pair[:], AF.Exp)
                nc.tensor.matmul(U[:, 0:512], vt_pair(p, bi),
                                 ex[:].rearrange("p (g t) -> p g t", g=2),
                                 start=False, stop=(p == 15), perf_mode=DR)

            def emit_halo(bi, uh):
                exh = ep.tile([128, 64], f8e5, tag="exh")
                nc.scalar.activation(exh[:], halo[:, 64 * bi:64 * bi + 64], AF.Exp)
                for g in range(32):
                    nc.tensor.matmul(uh, vt_chunk(g, bi),
                                     exh[:, 2 * g:2 * g + 2],
                                     start=False, stop=(g == 31))

            def emit_z(bi, U, W3, psUx):
                uh = halo[:, 128 + 2 * bi:130 + 2 * bi]
                rU = wp.tile([1, QN], f32, tag="rU")
                nc.vector.reciprocal(rU[:, 0:512], U[32:33, :])
                nc.vector.reciprocal(rU[:, 512:QN], uh[32:33, :])
                rUb = wp.tile([16, QN], f32, tag="rUb")
                nc.gpsimd.partition_broadcast(rUb[:], rU[:])
                AVn = wp.tile([16, QN], f32, tag="AVn")
                nc.vector.tensor_tensor(AVn[:, 0:512], U[0:16, :], rUb[:, 0:512],
                                        op=ALU.mult)
                nc.vector.tensor_tensor(AVn[:, 512:QN], uh[0:16, :],
                                        rUb[:, 512:QN], op=ALU.mult)
                Z = wp.tile([17, QN], bf16, tag="Z")
                nc.scalar.activation(Z[0:16, :], AVn[:], AF.Exp)
                dn1 = psUx.tile([1, 512], f32, tag="UA" if bi == 0 else "UB")
                dn2 = halo[0:1, 144:146]
                nc.tensor.matmul(dn1[:], ones16, Z[0:16, 0:512], start=True,
                                 stop=True)
                nc.tensor.matmul(dn2, ones16, Z[0:16, 512:QN], start=True,
                                 stop=True)
                rd = wp.tile([1, QN], f32, tag="rd")
                nc.vector.reciprocal(rd[:, 0:512], dn1[:])
                nc.vector.reciprocal(rd[:, 512:QN], dn2)
                dnb = wp.tile([1, QN], bf16, tag="dnb")
                nc.vector.tensor_copy(dnb[:, 0:512], dn1[:])
                nc.vector.tensor_copy(dnb[:, 512:QN], dn2)
                nc.sync.dma_start(Z[16:17, :], dnb[:])
                o31 = psUx.tile([16, 512], f32, tag="UA" if bi == 0 else "UB")
                o32 = halo[0:16, 148:150]
                nc.tensor.matmul(o31[:], W3, Z[:, 0:512], start=True, stop=True)
                nc.tensor.matmul(o32, W3, Z[:, 512:QN], start=True, stop=True)
                rdb = wp.tile([16, QN], f32, tag="rdb")
                nc.gpsimd.partition_broadcast(rdb[:], rd[:])
                wavm = wp.tile([16, QN], f32, tag="wavm")
                nc.vector.tensor_tensor(wavm[:, 0:512], o31[:], rdb[:, 0:512],
                                        op=ALU.mult)
                nc.vector.tensor_tensor(wavm[:, 512:QN], o32, rdb[:, 512:QN],
                                        op=ALU.mult)
                wavc = wp.tile([16, QN], bf16, tag="wavc")
                nc.vector.tensor_scalar(wavc[:], wavm[:], 0.0, 6.0,
                                        ALU.max, ALU.min)
                nc.sync.dma_start(y48[16 + 16 * bi:32 + 16 * bi, :], wavc[:])

            uhA = halo[0:33, 128:130]
            uhB = halo[0:33, 130:132]
            emit_eps(UA, uhA)
            emit_eps(UB, uhB)

            c0w = [CONVW[0:49, 120 * dw:120 * dw + 120] for dw in range(3)]
            c1w = [CONVW[0:121, 360 + 60 * dw:360 + 60 * dw + 60] for dw in range(4)]
            c2w = [CONVW[0:61, 600 + 30 * dw:600 + 30 * dw + 30] for dw in range(4)]
            c0ps = psS.tile([120, 512], f32, tag="S")

            STAG = 5
            for p in range(16 + STAG):
                if p < 16:
                    emit_pair(0, UA, p)
                if p == 15:
                    emit_halo(0, uhA)
                if p >= STAG:
                    emit_pair(1, UB, p - STAG)
                if p == 16:
                    emit_z(0, UA, W3A, psUA)
            # conv0 rows 0:32 can run while B's tail (halo/z) is in flight
            for dw in range(3):
                nc.tensor.matmul(c0ps[:], c0w[dw][0:32, :], y48[0:32, dw:dw + 512],
                                 start=(dw == 0), stop=False)
            emit_halo(1, uhB)
            emit_z(1, UB, W3B, psUB)

            # ---- 9. conv stack
            y0 = cp.tile([121, 516], bf16, tag="y0")
            for dw in range(3):
                nc.tensor.matmul(c0ps[:], c0w[dw][32:49, :], y48[32:49, dw:dw + 512],
                                 start=False, stop=(dw == 2))
            nc.vector.tensor_scalar(y0[0:120, 0:512], c0ps[:], 0.0, 6.0,
                                    ALU.max, ALU.min)
            nc.sync.dma_start(y0[120:121, 0:512], a['CONSTX'][16:17, 244:756])
            y1 = cp.tile([61, 132], bf16, tag="y1")
            c1ps = psS.tile([60, 128], f32, tag="S")
            for dw in range(4):
                rhs = y0[:, dw:dw + 4 * 128].rearrange("p (n s) -> p n s", s=4)[:, :, 0]
                nc.tensor.matmul(c1ps[:], c1w[dw], rhs, start=(dw == 0), stop=(dw == 3))
            nc.vector.tensor_scalar(y1[0:60, 0:128], c1ps[:], 0.0, 6.0,
                                    ALU.max, ALU.min)
            nc.sync.dma_start(y1[60:61, 0:128], a['CONSTX'][16:17, 244:372])
            y2 = wp.tile([30, 32], f32, tag="y2")
            c2ps = psS.tile([30, 32], f32, tag="S")
            for dw in range(4):
                rhs = y1[:, dw:dw + 4 * 32].rearrange("p (n s) -> p n s", s=4)[:, :, 0]
                nc.tensor.matmul(c2ps[:], c2w[dw], rhs, start=(dw == 0), stop=(dw == 3))
            nc.vector.tensor_scalar(y2[:], c2ps[:], 0.0, 6.0, ALU.max, ALU.min)

            # ---- 10. AllGather conv2 maps, then the head on every core
            ib = dp.tile([30, 32], f32)
            ob = dp.tile([NC, 30, 32], f32)
            nc.sync.dma_start(ib[:], y2[:])
            nc.gpsimd.collective_compute(
                "AllGather", ALU.bypass,
                replica_groups=[list(range(NC))],
                ins=[ib.opt()], outs=[ob.opt()])
            y2a = cp.tile([31, 256], f32, tag="y2a")
            nc.sync.dma_start(y2a[0:30, :].rearrange("r (i c) -> r i c", i=NC),
                              ob.rearrange("i r c -> r i c"))
            nc.sync.dma_start(y2a[30:31, :], a['HONES'][:])

            c3ps = psS.tile([15, 84], f32, tag="S")
            for dw in range(4):
                rhs = y2a[0:31, dw:dw + 3 * 84].rearrange(
                    "p (n s) -> p n s", s=3)[:, :, 0]
                nc.tensor.matmul(c3ps[:], HEADW[:, 15 * dw:15 * dw + 15], rhs,
                                 start=(dw == 0), stop=(dw == 3))
            y3 = wp.tile([15, 84], f32, tag="y3")
            nc.vector.tensor_scalar(y3[:], c3ps[:], 0.0, 6.0, ALU.max, ALU.min)
            nc.sync.dma_start(a['scr'][:], y3[:])
            y42T = cp.tile([31, 42], f32, tag="y42T")
            flat = a['scr'].rearrange("a b -> (a b)").rearrange("(r m) -> m r", m=30)
            nc.sync.dma_start(y42T[0:30, :], flat)
            nc.sync.dma_start(y42T[30:31, :], a['HONES'][0:1, 0:42])
            p1 = psS.tile([15, 42], f32, tag="S")
            nc.tensor.matmul(p1[:], HEADW[:, 60:75], y42T[:], start=True, stop=True)
            e1 = wp.tile([15, 42], f32, tag="e1")
            nc.scalar.activation(e1[:], p1[:], AF.Exp, scale=-1.0)
            h = cp.tile([16, 42], f32, tag="h")
            nc.vector.tensor_scalar(h[0:15, :], e1[:], 1.0, None, ALU.add)
            nc.vector.reciprocal(h[0:15, :], h[0:15, :])
            nc.sync.dma_start(h[15:16, :], a['HONES'][0:1, 0:42])
            p2 = psS.tile([2, 42], f32, tag="S")
            nc.tensor.matmul(p2[:], HEADW[0:16, 75:77], h[:], start=True, stop=True)
            e2 = wp.tile([2, 42], f32, tag="e2")
            nc.scalar.activation(e2[:], p2[:], AF.Exp, scale=-1.0)
            e2p = wp.tile([2, 42], f32, tag="e2p")
            nc.vector.tensor_scalar(e2p[:], e2[:], 1.0, None, ALU.add)
            o = wp.tile([2, 42], f32, tag="o")
            nc.vector.reciprocal(o[:], e2p[:])
            nc.sync.dma_start(a['out'].rearrange("r c -> c r"), o[:])
    nc.compile()
    return nc


_NC1 = None


def _ensure_built():
    global _NC1
    if _NC1 is None:
        _NC1 = _build()


def _run_spmd_retry(nc, in_maps, core_ids, trace, trace_cores=None, tries=3):
    import time
    last = None
    for attempt in range(tries):
        try:
            return run_bass_kernel_spmd(nc, in_maps, core_ids, trace=trace,
                                        trace_cores=trace_cores)
        except Exception as e:  # transient accelerator errors observed (~10%)
            last = e
            time.sleep(2.0 * (attempt + 1))
    raise last


def _run(inputs, trace=False, trace_cores=None):
    _ensure_built()
    c = build_consts(**inputs)
    shared = {k: c[k] for k in ('ETW', 'EWB', 'CONVW', 'F8', 'F8E5', 'F8B',
                                'HEADW', 'HONES')}
    in_maps = [{**shared, 'CONSTX': c['CONSTX'][ci]} for ci in range(NC)]
    res1 = _run_spmd_retry(_NC1, in_maps, list(range(NC)), trace, trace_cores)
    out = np.asarray(res1.results[0]['out'], np.float32)
    return out, res1, None


def kernel(**inputs) -> np.ndarray:
    out, _, _ = _run(inputs, trace=False)
    return out
